# revision 3
# baseline (speedup 1.0000x reference)
"""Fused global pooling (mean/max/std over H*W per channel) + tiny MLP.

Input x: [1024, 1024, 384] f32. Sharded along H across 8 NeuronCores
(128 H-rows each). Each core computes per-channel partial sum / sumsq
(via ones-matmul on PE, f32r) and per-partition max (DVE); host combines
the 8 partial stats, finishes mean/std/max, and runs the 3-layer MLP.
"""
import os
import sys

sys.path.insert(0, "/opt/trn_rl_repo")

import numpy as np

H, W, C = 1024, 1024, 384
N_CORES = 8
P = 128                      # SBUF partitions; also H-rows per core
ROW = W * C                  # elements per H-row = 393216
FD = int(os.environ.get("BASS_KERNEL_FD", "6144"))  # free elems/tile/partition
T = ROW // FD                # tiles per core
R = FD // C                  # channel-groups per tile
HALF = FD > 6144             # split ACT square into halves to bound SBUF

_CACHE: dict = {}

# set by test.py via env to collect a perfetto trace + HW exec time
TRACE = bool(int(os.environ.get("BASS_KERNEL_TRACE", "0")))
last_result = None           # BassKernelResults of the most recent run


def _build():
    import concourse.bacc as bacc
    import concourse.mybir as mybir
    import concourse.tile as tile

    f32 = mybir.dt.float32
    f32r = mybir.dt.float32r

    nc = bacc.Bacc(trn_type="TRN2")
    x_in = nc.declare_dram_parameter("x", [P, ROW], f32, isOutput=False)
    out_max = nc.declare_dram_parameter("out_max", [P, C], f32, isOutput=True)
    out_sum = nc.declare_dram_parameter("out_sum", [1, C], f32, isOutput=True)
    out_sq = nc.declare_dram_parameter("out_sq", [1, C], f32, isOutput=True)

    ACCW = min(FD, 6144)         # max-accumulator width
    SQW = FD // 2 if HALF else FD  # ACT square chunk width
    NH = FD // SQW               # halves per tile
    RH = SQW // C                # channel-groups per half

    with tile.TileContext(nc) as tc:
        with (
            tc.tile_pool(name="x", bufs=2 if HALF else 3) as xpool,
            tc.tile_pool(name="sq", bufs=2) as sqpool,
            tc.tile_pool(name="acc", bufs=1) as accpool,
            tc.tile_pool(name="misc", bufs=1) as misc,
            tc.tile_pool(name="psum", bufs=1, space="PSUM") as psum_pool,
        ):
            ones_f = misc.tile([P, 1], f32)
            nc.vector.memset(ones_f[:], 1.0)
            ones = misc.tile([P, 1], f32r)
            nc.vector.tensor_copy(ones[:], ones_f[:])

            acc_max = accpool.tile([P, ACCW], f32)
            nc.vector.memset(acc_max[:], float("-inf"))

            ps_sum = psum_pool.tile([1, C], f32)
            ps_sq = psum_pool.tile([1, C], f32)

            for t in range(T):
                xt = xpool.tile([P, FD], f32r)
                nc.sync.dma_start(xt[:], x_in[:, t * FD:(t + 1) * FD].bitcast(f32r))

                for h in range(NH):
                    hsl = slice(h * SQW, (h + 1) * SQW)
                    nc.vector.tensor_max(
                        acc_max[:, 0:SQW], acc_max[:, 0:SQW], xt[:, hsl].bitcast(f32)
                    )
                    sq = sqpool.tile([P, SQW], f32r)
                    nc.scalar.square(sq[:], xt[:, hsl].bitcast(f32))

                    for r in range(RH):
                        g = h * RH + r
                        st = (t == 0) and (g == 0)
                        sp = (t == T - 1) and (g == R - 1)
                        xsl = slice(g * C, (g + 1) * C)
                        ssl = slice(r * C, (r + 1) * C)
                        nc.tensor.matmul(
                            ps_sum[:], ones[:], xt[:, xsl], start=st, stop=sp
                        )
                        nc.tensor.matmul(
                            ps_sq[:], ones[:], sq[:, ssl], start=st, stop=sp
                        )

            # log-step fold of the channel-group max columns into group 0
            w = ACCW // 2
            while w >= C:
                nc.vector.tensor_max(
                    acc_max[:, 0:w], acc_max[:, 0:w], acc_max[:, w:2 * w]
                )
                w //= 2
            nc.sync.dma_start(out_max[:], acc_max[:, 0:C])

            row_sum = misc.tile([1, C], f32)
            nc.vector.tensor_copy(row_sum[:], ps_sum[:])
            nc.sync.dma_start(out_sum[:], row_sum[:])
            row_sq = misc.tile([1, C], f32)
            nc.vector.tensor_copy(row_sq[:], ps_sq[:])
            nc.sync.dma_start(out_sq[:], row_sq[:])

    nc.compile()
    return nc


def kernel(x, W1, b1, W2, b2, W3, b3):
    global last_result
    from concourse.bass_utils import run_bass_kernel_spmd

    if "nc" not in _CACHE:
        _CACHE["nc"] = _build()
    nc = _CACHE["nc"]

    x = np.ascontiguousarray(np.asarray(x, dtype=np.float32))
    assert x.shape == (H, W, C)

    core_ids = list(range(N_CORES))
    in_maps = [
        {"x": x[k * P:(k + 1) * P].reshape(P, ROW)} for k in core_ids
    ]
    res = run_bass_kernel_spmd(nc, in_maps, core_ids, trace=TRACE)
    last_result = res

    n = H * W
    sums = np.zeros(C, dtype=np.float64)
    sqs = np.zeros(C, dtype=np.float64)
    mx = np.full(C, -np.inf, dtype=np.float64)
    for k in core_ids:
        r = res.results[k]
        sums += r["out_sum"][0].astype(np.float64)
        sqs += r["out_sq"][0].astype(np.float64)
        mx = np.maximum(mx, r["out_max"].astype(np.float64).max(axis=0))

    mean = sums / n
    var = (sqs - n * mean * mean) / (n - 1)
    std = np.sqrt(np.maximum(var, 0.0))

    feats = np.concatenate([mean, mx, std])
    h = np.maximum(feats @ np.asarray(W1, np.float64) + np.asarray(b1, np.float64), 0.0)
    h = np.maximum(h @ np.asarray(W2, np.float64) + np.asarray(b2, np.float64), 0.0)
    logits = h @ np.asarray(W3, np.float64) + np.asarray(b3, np.float64)
    e = np.exp(logits - logits.max())
    return (e / e.sum()).astype(np.float32)


# revision 6
# speedup vs baseline: 1.0190x; 1.0190x over previous
"""Fused global pooling (mean/max/std over H*W per channel) + tiny MLP.

Input x: [1024, 1024, 384] f32. Sharded along H across 8 NeuronCores
(128 H-rows each). Each core computes per-channel partial sum / sumsq
(via ones-matmul on PE, f32r) and per-partition max (DVE); host combines
the 8 partial stats, finishes mean/std/max, and runs the 3-layer MLP.
"""
import os
import sys

sys.path.insert(0, "/opt/trn_rl_repo")

import numpy as np

H, W, C = 1024, 1024, 384
N_CORES = 8
P = 128                      # SBUF partitions; also H-rows per core
ROW = W * C                  # elements per H-row = 393216
FD = int(os.environ.get("BASS_KERNEL_FD", "6144"))  # free elems/tile/partition
T = ROW // FD                # tiles per core
R = FD // C                  # channel-groups per tile
HALF = FD > 6144             # split ACT square into halves to bound SBUF

_CACHE: dict = {}

# set by test.py via env to collect a perfetto trace + HW exec time
TRACE = bool(int(os.environ.get("BASS_KERNEL_TRACE", "0")))
last_result = None           # BassKernelResults of the most recent run


def _build():
    import concourse.bacc as bacc
    import concourse.mybir as mybir
    import concourse.tile as tile

    f32 = mybir.dt.float32
    f32r = mybir.dt.float32r

    nc = bacc.Bacc(trn_type="TRN2")
    x_in = nc.declare_dram_parameter("x", [P, ROW], f32, isOutput=False)
    out_max = nc.declare_dram_parameter("out_max", [P, C], f32, isOutput=True)
    out_sum = nc.declare_dram_parameter("out_sum", [1, C], f32, isOutput=True)
    out_sq = nc.declare_dram_parameter("out_sq", [1, C], f32, isOutput=True)

    SQW = FD // 4 if HALF else FD  # ACT square chunk width
    ACCW = SQW                   # max-accumulator width
    NH = FD // SQW               # halves per tile
    RH = SQW // C                # channel-groups per half

    with tile.TileContext(nc) as tc:
        with (
            tc.tile_pool(name="x", bufs=3 if HALF else 4) as xpool,
            tc.tile_pool(name="sq", bufs=2) as sqpool,
            tc.tile_pool(name="acc", bufs=1) as accpool,
            tc.tile_pool(name="misc", bufs=1) as misc,
            tc.tile_pool(name="psum", bufs=1, space="PSUM") as psum_pool,
        ):
            ones_f = misc.tile([P, 1], f32)
            nc.vector.memset(ones_f[:], 1.0)
            ones = misc.tile([P, 1], f32r)
            nc.vector.tensor_copy(ones[:], ones_f[:])

            acc_max = accpool.tile([P, ACCW], f32)
            nc.vector.memset(acc_max[:], float("-inf"))

            ps_sum = psum_pool.tile([1, C], f32)
            ps_sq = psum_pool.tile([1, C], f32)

            for t in range(T):
                xt = xpool.tile([P, FD], f32r)
                nc.sync.dma_start(xt[:], x_in[:, t * FD:(t + 1) * FD].bitcast(f32r))

                for h in range(NH):
                    hsl = slice(h * SQW, (h + 1) * SQW)
                    nc.vector.tensor_max(
                        acc_max[:, 0:SQW], acc_max[:, 0:SQW], xt[:, hsl].bitcast(f32)
                    )
                    sq = sqpool.tile([P, SQW], f32r)
                    nc.scalar.square(sq[:], xt[:, hsl].bitcast(f32))

                    for r in range(RH):
                        g = h * RH + r
                        st = (t == 0) and (g == 0)
                        sp = (t == T - 1) and (g == R - 1)
                        xsl = slice(g * C, (g + 1) * C)
                        ssl = slice(r * C, (r + 1) * C)
                        nc.tensor.matmul(
                            ps_sum[:], ones[:], xt[:, xsl], start=st, stop=sp
                        )
                        nc.tensor.matmul(
                            ps_sq[:], ones[:], sq[:, ssl], start=st, stop=sp
                        )

            # log-step fold of the channel-group max columns into group 0
            w = ACCW // 2
            while w >= C:
                nc.vector.tensor_max(
                    acc_max[:, 0:w], acc_max[:, 0:w], acc_max[:, w:2 * w]
                )
                w //= 2
            nc.sync.dma_start(out_max[:], acc_max[:, 0:C])

            row_sum = misc.tile([1, C], f32)
            nc.vector.tensor_copy(row_sum[:], ps_sum[:])
            nc.sync.dma_start(out_sum[:], row_sum[:])
            row_sq = misc.tile([1, C], f32)
            nc.vector.tensor_copy(row_sq[:], ps_sq[:])
            nc.sync.dma_start(out_sq[:], row_sq[:])

    nc.compile()
    return nc


def kernel(x, W1, b1, W2, b2, W3, b3):
    global last_result
    from concourse.bass_utils import run_bass_kernel_spmd

    if "nc" not in _CACHE:
        _CACHE["nc"] = _build()
    nc = _CACHE["nc"]

    x = np.ascontiguousarray(np.asarray(x, dtype=np.float32))
    assert x.shape == (H, W, C)

    core_ids = list(range(N_CORES))
    in_maps = [
        {"x": x[k * P:(k + 1) * P].reshape(P, ROW)} for k in core_ids
    ]
    res = run_bass_kernel_spmd(nc, in_maps, core_ids, trace=TRACE)
    last_result = res

    n = H * W
    sums = np.zeros(C, dtype=np.float64)
    sqs = np.zeros(C, dtype=np.float64)
    mx = np.full(C, -np.inf, dtype=np.float64)
    for k in core_ids:
        r = res.results[k]
        sums += r["out_sum"][0].astype(np.float64)
        sqs += r["out_sq"][0].astype(np.float64)
        mx = np.maximum(mx, r["out_max"].astype(np.float64).max(axis=0))

    mean = sums / n
    var = (sqs - n * mean * mean) / (n - 1)
    std = np.sqrt(np.maximum(var, 0.0))

    feats = np.concatenate([mean, mx, std])
    h = np.maximum(feats @ np.asarray(W1, np.float64) + np.asarray(b1, np.float64), 0.0)
    h = np.maximum(h @ np.asarray(W2, np.float64) + np.asarray(b2, np.float64), 0.0)
    logits = h @ np.asarray(W3, np.float64) + np.asarray(b3, np.float64)
    e = np.exp(logits - logits.max())
    return (e / e.sum()).astype(np.float32)
